# revision 46
# baseline (speedup 1.0000x reference)
"""Trainium2 Bass kernel for a transformer decoder block (self-attn + cross-attn + FFN).

Sharding: zero-collective data parallelism over tokens. 8 cores; core c handles
batch b = c//4 and the 4 query blocks {4s + (c%4) : s in 0..3} of 128 tokens
each. Each core redundantly computes full-sequence K/V projections and
everything else only for its own tokens. One SPMD program for all cores; the
causal structure is j-uniform (compute block (s, kl) iff kl <= 4s+3, the
superset over j) with per-core additive masks as input data.

Design (vs the all-bf16 predecessor):
- the K/V/Q/O projection matmuls run fp8e4m3 in DoubleRow perf mode (two
  contraction rows per partition -> half the PE row-cycles of bf16), with
  two surgical bf16 exceptions driven by error analysis:
  * the FFN stays bf16 end to end: 2.4%-RMS fp8 noise on the hidden /
    operands random-walks through the 1024/4096-term contractions into
    ~1.5-3% output std -- the dominant error mode (measured, not guessed).
  * the causal early tokens see AV ~= v directly (no key averaging), so
    SA's V for key block 0 runs bf16 off a bf16 transpose copy, and SA's
    O-projection (avall + Wo) stays bf16. Every other fp8 error source is
    buried by softmax averaging over >=128 keys.
- the attention core (scores, exp, AV, softmax normalize) stays bf16; PSUM
  accumulation and the residual stream stay fp32. (Scores/AV gain nothing
  from fp8 anyway: PE cost is out-free-size cycles regardless of the
  contraction width, and their per-instruction contraction is already <=128.)
- fp8 subnormal dodge: fp8 weights are host-packed premultiplied by 2^6
  (their native 0.02 scale would land in fp8 subnormals). The inflation
  cancels inside existing ACT scale params: exp(scale) absorbs 2^-12 (q and
  k both inflated), the CA O-proj evict absorbs 2^-12, the SA one 2^-6.
  Zero extra instructions. The x64 V also parks the normalized attention
  outputs at ~1 scale = prime fp8 territory. The FFN hidden is inflated x8
  through the Relu evict (b1 host-premultiplied) for the same reason.
- full-T processing; V kept SBUF-resident per (key block, pair) as
  [V_A(64) | 1 | V_B(64) | 1]: the ones columns ride the AV matmuls so each
  head's softmax denominator lands on PSUM row 64 for free.
- causal skipping: blocks with kl > 4s+3 never computed; each visited kl
  gets a MULTIPLICATIVE per-core mask (0 / 1 / tril) applied on DVE to the
  post-exp weights of its own block s = kl//4.
- packed O-projection: normalized head A in rows 0:64 and head B rows 64:128
  of a shared fp8 tile (placed by an SBUF->SBUF DMA partition shift), giving
  a 128-wide DoubleRow contraction over pair pairs.
- software-pipelined attention: scores(kl+1) are emitted before AV(kl);
  next pair's K/Q projection groups drain one-per-kl as ACT-catchup filler.
- the LN+transpose streams interleave the V projection per token block;
  transposes run bf16 through bitcast PSUM views, the evict converts to fp8.
- O-proj / FFN-W2 transposes are pipelined one m-chunk behind the matmuls.
- weights host-packed into exact SBUF layout: every weight DMA is a fully
  contiguous copy.
"""
import sys
import numpy as np
import ml_dtypes

for _p in ('/opt/trn_rl_repo',):
    if _p not in sys.path:
        sys.path.append(_p)

import concourse.bass as bass
import concourse.tile as tile
from concourse import bacc, mybir
from concourse.masks import make_identity

P = 128
HD = 64
EPS = 1e-5
NEG = -1e9
WSCALE = 64.0              # weight inflation at pack time (2^6)
SC_W = 1.0 / WSCALE        # evict compensation for one inflated operand
SC_WW = SC_W * SC_W        # ... for two (q*k, Wo*AV)
RSCALE = 8.0               # FFN-hidden inflation (b1 host-premultiplied)

f32 = mybir.dt.float32
f32r = mybir.dt.float32r
bf16 = mybir.dt.bfloat16
f8 = mybir.dt.float8e4
AF = mybir.ActivationFunctionType
DR = mybir.MatmulPerfMode.DoubleRow
NP_F8 = ml_dtypes.float8_e4m3


class Cfg:
    def __init__(self, T=2048, D=1024, H=16, FF=4096):
        self.T, self.D, self.H, self.FF = T, D, H, FF
        self.OWN = T // 4          # tokens per core
        self.NQB = self.OWN // P   # own q-blocks (128 each)
        self.DC = D // P           # D chunks
        self.FC = FF // P          # FFN chunks
        self.PAIRS = H // 2
        self.KB = T // P           # key blocks (global)
        self.VCH = (H * HD + 511) // 512


def build_nc(cfg, with_gb):
    T, D, H, FF = cfg.T, cfg.D, cfg.H, cfg.FF
    OWN, NQB, DC, FC = cfg.OWN, cfg.NQB, cfg.DC, cfg.FC
    PAIRS, KB, VCH = cfg.PAIRS, cfg.KB, cfg.VCH
    DC2, FC2, PR2 = DC // 2, FC // 2, PAIRS // 2
    scale = float(D) ** -0.5

    nc = bacc.Bacc("TRN2", target_bir_lowering=False, debug=False)
    dp = nc.declare_dram_parameter
    x_dec = dp("x_dec", [T, D], f32, isOutput=False)
    x_enc = dp("x_enc", [T, D], f32, isOutput=False)
    x_own = dp("x_own", [OWN, D], f32, isOutput=False)
    wq_sa = dp("wq_sa", [P, PAIRS * DC * P], f8, isOutput=False)
    wk_sa = dp("wk_sa", [P, PAIRS * DC * P], f8, isOutput=False)
    wv_sa = dp("wv_sa", [P, VCH * DC * 512], bf16, isOutput=False)
    wv_sa8 = dp("wv_sa8", [P, VCH * DC * 512], f8, isOutput=False)
    wo_sa = dp("wo_sa", [P, DC * PAIRS * P], bf16, isOutput=False)
    bo_sa = dp("bo_sa", [D], f32, isOutput=False)
    wq_ca = dp("wq_ca", [P, PAIRS * DC * P], f8, isOutput=False)
    wk_ca = dp("wk_ca", [P, PAIRS * DC * P], f8, isOutput=False)
    wv_ca = dp("wv_ca", [P, VCH * DC * 512], f8, isOutput=False)
    wo_ca = dp("wo_ca", [P, DC * PAIRS * P], f8, isOutput=False)
    bo_ca = dp("bo_ca", [D], f32, isOutput=False)
    w1 = dp("w1", [P, FC * DC * P], bf16, isOutput=False)
    b1 = dp("b1", [FF], f32, isOutput=False)
    w2 = dp("w2", [P, DC * FC * P], bf16, isOutput=False)
    b2 = dp("b2", [D], f32, isOutput=False)
    masks = dp("masks", [P, KB * P], bf16, isOutput=False)
    gbs = {}
    if with_gb:
        for n in ("g1", "be1", "g2", "be2", "g3", "be3"):
            gbs[n] = dp(n, [D], f32, isOutput=False)
    out = dp("out", [OWN, D], f32, isOutput=True)

    # j-uniform causal-skip table: visit (s, kl) iff kl <= 4s+3
    def s0_of(kl):
        return max(0, -(-(kl - (NQB - 1)) // NQB))

    from contextlib import ExitStack
    with tile.TileContext(nc) as tc:
        with ExitStack() as _ctx:
            _ctx.enter_context(nc.allow_low_precision(
                reason="fp8 matmul operands, fp32 accumulation"))
            _pool = lambda nm, bufs, **kw: _ctx.enter_context(
                tc.tile_pool(name=nm, bufs=bufs, **kw))
            constp = _pool("constp", 1)
            bp = _pool("bp", 1)
            ldbp = _pool("ldbp", 3)
            xTp = _pool("xTp", 1)
            vsbp = _pool("vsbp", 1)
            ktp = _pool("ktp", 2)
            qtp = _pool("qtp", 2)
            avkp = _pool("avkp", 1)
            x0p = _pool("x0p", 1)
            lnqp = _pool("lnqp", 1)
            rtp = _pool("rtp", 1)
            residp = _pool("residp", 4)
            wp = _pool("wp", 2)
            wvp = _pool("wvp", 1)
            wop = _pool("wop", 2)
            w2p = _pool("w2p", 2)
            smallp = _pool("smallp", 8)
            pbp = _pool("pbp", 2)
            evp = _pool("evp", 2)
            normp = _pool("normp", 1)
            tmpp = _pool("tmpp", 1)
            linps = _pool("linps", 2, space="PSUM")
            scps = _pool("scps", 2, space="PSUM")
            avps = _pool("avps", 2, space="PSUM")

            # ---------------- constants ----------------
            identb = constp.tile([P, P], bf16, tag="identb")
            make_identity(nc, identb[:])
            onesf = constp.tile([P, HD], f32, tag="onesf")
            nc.any.memset(onesf[:], 1.0)
            onesr = constp.tile([P, HD], f32r, tag="onesr")
            nc.vector.tensor_copy(onesr[:], onesf[:])
            epst = constp.tile([P, 1], f32, tag="epst")
            nc.any.memset(epst[:], EPS)
            maskt = constp.tile([P, KB * P], bf16, tag="maskt")
            nc.sync.dma_start(maskt[:], masks[:])
            ro = lambda ap: ap.bitcast(f32r)

            def load_bias_T(dram, n):
                t = bp.tile([P, n], f32, tag=dram.tensor.name + "_t")
                nc.sync.dma_start(t[:], dram[:].rearrange("(d p) -> p d", p=P))
                return t

            gb_tiles = {}
            if with_gb:
                for gk, bk, key in (("g1", "be1", 1), ("g2", "be2", 2),
                                    ("g3", "be3", 3)):
                    gb_tiles[key] = (load_bias_T(gbs[gk][:], DC),
                                     load_bias_T(gbs[bk][:], DC))
            bo_sa_t = load_bias_T(bo_sa[:], DC)
            bo_ca_t = load_bias_T(bo_ca[:], DC)
            b1_t = load_bias_T(b1[:], FC)
            b2_t = load_bias_T(b2[:], DC)

            xT_cur = [None]

            # ---------------- helpers ----------------
            def emit_ln(xt, pre_st6=None):
                """LayerNorm stats+apply over D -> new bf16 tile [P, D].
                gamma/beta (if nontrivial) fold in at transpose-evict.
                pre_st6: stats already accumulated chunk-wise upstream."""
                if pre_st6 is None:
                    nch = (D + 511) // 512
                    st6 = smallp.tile([P, nch * 6], f32, tag="st6")
                    for i in range(nch):
                        c0, c1 = i * 512, min(D, (i + 1) * 512)
                        nc.vector.bn_stats(st6[:, i * 6:(i + 1) * 6],
                                           xt[:, c0:c1])
                else:
                    st6 = pre_st6
                mv = smallp.tile([P, 2], f32, tag="mv")
                nc.vector.bn_aggr(mv[:], st6[:].rearrange("p (a b) -> p a b",
                                                          b=6))
                std = smallp.tile([P, 1], f32, tag="std")
                nc.scalar.activation(std[:], mv[:, 1:2], AF.Sqrt, bias=epst[:])
                rstd = smallp.tile([P, 1], f32, tag="rstd")
                nc.vector.reciprocal(rstd[:], std[:])
                mrs = smallp.tile([P, 1], f32, tag="mrs")
                nc.vector.tensor_mul(mrs[:], mv[:, 0:1], rstd[:])
                nmrs = smallp.tile([P, 1], f32, tag="nmrs")
                nc.vector.tensor_scalar_mul(nmrs[:], mrs[:], -1.0)
                lnt = ldbp.tile([P, D], bf16, tag="ldb")
                nc.scalar.activation(lnt[:], xt[:], AF.Identity,
                                     bias=nmrs[:], scale=rstd[:])
                return lnt

            def emit_transposes(src, dst_view, gbkey, ev_eng=nc.vector):
                """Transpose bf16 [P, D] src into dst_view [P, DC, P]
                (d-major, fp8 dest); all DC transposes through one
                bf16-bitcast PSUM tile, the evict converts to fp8."""
                ps = linps.tile([P, 512], f32, tag="lin")
                psb = ps[:].bitcast(bf16)          # [P, 1024] bf16 view
                for d in range(DC):
                    nc.tensor.transpose(psb[:, d * P:(d + 1) * P],
                                        src[:, d * P:(d + 1) * P],
                                        identb[:])
                if with_gb and gbkey is not None:
                    gt, bt = gb_tiles[gbkey]
                    for d in range(DC):
                        nc.scalar.activation(
                            dst_view[:, d, :], psb[:, d * P:(d + 1) * P],
                            AF.Identity, bias=bt[:, d:d + 1],
                            scale=gt[:, d:d + 1])
                else:
                    ev_eng.tensor_copy(dst_view[:, :, :],
                                       psb[:].rearrange(
                                           "p (d c) -> p d c", c=P))

            def stream_to_xT(src_dram, gbkey, wv_dram, which,
                             wvb_dram=None):
                """Stream [T, D] from DRAM (cast to bf16 in the DMA),
                optional LN, transpose into a [P, DC*T] fp8 mega-tile; K/V
                projections run fp8 DoubleRow. If wvb_dram is given (the SA
                stream), token block 0's V runs bf16 off a bf16 copy of its
                transposes: causal tokens 0..127 attend only block 0, so
                they can't absorb fp8 V noise; every later token averages
                >=128 keys, which buries it. The V projection for each token
                block is interleaved right after its transposes so PE chews
                V work while the LN chain produces the next block."""
                xT = xTp.tile([P, DC * T], f8, tag="xT")
                xTv = xT[:].rearrange("p (d t) -> p d t", t=T)
                vsb = vsbp.tile([P, KB * PAIRS * 130], bf16, tag="vsb")
                v4 = vsb[:].rearrange("p (k r h c) -> p k r h c", r=PAIRS,
                                      h=2, c=65)
                nc.any.memset(v4[:, :, :, :, 64:65], 1.0)
                wvh = wvp.tile([P, VCH * DC * 512], f8, tag="wvh",
                               name=f"wvh_{which}")
                wvv = wvh[:].rearrange("p (n d c) -> p n d c", d=DC, c=512)
                for nch in range(VCH):
                    nc.sync.dma_start(
                        wvh[:, nch * DC * 512:(nch + 1) * DC * 512],
                        wv_dram[:, nch * DC * 512:(nch + 1) * DC * 512])
                if wvb_dram is not None:
                    wvb = wvp.tile([P, VCH * DC * 512], bf16, tag="wvb")
                    wvbv = wvb[:].rearrange("p (n d c) -> p n d c",
                                            d=DC, c=512)
                    x0T = x0p.tile([P, DC * P], bf16, tag="x0T")
                    x0Tv = x0T[:].rearrange("p (d t) -> p d t", t=P)
                    for nch in range(VCH):
                        nc.sync.dma_start(
                            wvb[:, nch * DC * 512:(nch + 1) * DC * 512],
                            wvb_dram[:, nch * DC * 512:(nch + 1) * DC * 512])
                def load(tb):
                    xt = ldbp.tile([P, D], bf16, tag="ldb",
                                   name=f"xt{tb}")
                    nc.gpsimd.dma_start(
                        xt[:], src_dram[tb * P:(tb + 1) * P, :])
                    return xt
                def emit_v(tb):
                    for nch in range(VCH):
                        ps = linps.tile([P, 512], f32, tag="lin",
                                        name=f"vps{tb}_{nch}")
                        if tb == 0 and wvb_dram is not None:
                            for d in range(DC):
                                nc.tensor.matmul(
                                    ps[:], x0Tv[:, d, :],
                                    wvbv[:, nch, d, :],
                                    start=(d == 0), stop=(d == DC - 1))
                        else:
                            for h in range(2):
                                for i in range(DC2):
                                    nc.tensor.matmul(
                                        ps[:, h * 256:(h + 1) * 256],
                                        xTv[:, 2 * i:2 * i + 2,
                                            tb * P:(tb + 1) * P],
                                        wvv[:, nch, 2 * i:2 * i + 2,
                                            h * 256:(h + 1) * 256],
                                        start=(i == 0), stop=(i == DC2 - 1),
                                        perf_mode=DR)
                        srcv = ps[:].rearrange("p (r two c) -> p r two c",
                                               two=2, c=HD)
                        dstv = v4[:, tb, 4 * nch:4 * nch + 4, :, 0:HD]
                        nc.scalar.activation(dstv[:, :, :, :], srcv[:],
                                             AF.Copy)
                nxt = load(0)
                for tb in range(T // P):
                    xt = nxt
                    if tb + 1 < T // P:
                        nxt = load(tb + 1)
                    lnt = emit_ln(xt) if gbkey is not None else xt
                    emit_transposes(lnt, xTv[:, :, tb * P:(tb + 1) * P],
                                    gbkey)
                    if tb == 0 and wvb_dram is not None:
                        emit_transposes(lnt, x0Tv[:, :, :], gbkey)
                    # V of the PREVIOUS block: its xT evict has landed, so
                    # the PE never waits on the evict it just scheduled
                    if tb >= 1:
                        emit_v(tb - 1)
                emit_v(T // P - 1)
                return xT, vsb

            def kq_steps(pair, which, wk_dram, wq_dram, rhs_of, kdt):
                """Emittable step closures that build kt/qt for `pair`;
                interleaved into the previous pair's kl loop so the PE fills
                ACT(exp)-pacing bubbles with projection work. kdt: dtype of
                the K weights / xT stream (fp8 -> DoubleRow)."""
                st = {}
                def s_dma():
                    wk = wp.tile([P, DC * P], kdt, tag="wx",
                                 name=f"wkt_{which}{pair}")
                    nc.sync.dma_start(
                        wk[:], wk_dram[:, pair * DC * P:(pair + 1) * DC * P])
                    wq = wp.tile([P, DC * P], f8, tag="wx",
                                 name=f"wqt_{which}{pair}")
                    nc.sync.dma_start(
                        wq[:], wq_dram[:, pair * DC * P:(pair + 1) * DC * P])
                    st['wk'], st['wq'] = wk, wq
                    st['kt'] = ktp.tile([P, T], bf16, tag="kt",
                                        name=f"kt_{which}{pair}")
                    st['qt'] = qtp.tile([P, OWN], bf16, tag="qt",
                                        name=f"qt_{which}{pair}")
                def s_kchunk(c):
                    def f():
                        wkv = st['wk'][:].rearrange("p (d c) -> p d c", c=P)
                        xv = xT_cur[0][:].rearrange("p (d t) -> p d t", t=T)
                        ps = linps.tile([P, 512], f32, tag="lin")
                        if kdt == f8:
                            for h in range(2):
                                for i in range(DC2):
                                    nc.tensor.matmul(
                                        ps[:, h * 256:(h + 1) * 256],
                                        wkv[:, 2 * i:2 * i + 2, :],
                                        xv[:, 2 * i:2 * i + 2,
                                           c * 512 + h * 256:
                                           c * 512 + (h + 1) * 256],
                                        start=(i == 0), stop=(i == DC2 - 1),
                                        perf_mode=DR)
                        else:
                            for d in range(DC):
                                nc.tensor.matmul(
                                    ps[:], wkv[:, d, :],
                                    xv[:, d, c * 512:(c + 1) * 512],
                                    start=(d == 0), stop=(d == DC - 1))
                        nc.vector.tensor_copy(st['kt'][:, c * 512:
                                                       (c + 1) * 512], ps[:])
                    return f
                def s_qblock(si):
                    def f():
                        wqv = st['wq'][:].rearrange("p (d c) -> p d c", c=P)
                        if si == 0:
                            st['qps'] = linps.tile([P, 512], f32, tag="lin",
                                                   name=f"qps_{which}{pair}")
                        for i in range(DC2):
                            nc.tensor.matmul(
                                st['qps'][:, si * P:(si + 1) * P],
                                wqv[:, 2 * i:2 * i + 2, :],
                                rhs_of(i, si),
                                start=(i == 0), stop=(i == DC2 - 1),
                                perf_mode=DR)
                        if si == NQB - 1:
                            nc.vector.tensor_copy(st['qt'][:], st['qps'][:])
                    return f
                steps = [s_dma] + [s_kchunk(c) for c in range(T // 512)] + \
                        [s_qblock(si) for si in range(NQB)]
                return steps, st

            def emit_attention(pair, qt, kt, vsb, causal, fill_steps, avall,
                               exp_scale, avdt):
                """One head pair's attention -> packed normalized fp8 columns
                pair*OWN:(pair+1)*OWN of avall. Software-pipelined:
                scores(kl+1) are emitted before AV(kl); `fill_steps` (next
                pair's K/Q projection groups) drain one per kl iteration."""
                avtA = avps.tile([P, OWN], f32, tag="avt",
                                 name=f"avtA_{causal}{pair}")
                avtB = avps.tile([P, OWN], f32, tag="avt",
                                 name=f"avtB_{causal}{pair}")
                fill = list(fill_steps)
                pbs = {}

                def emit_scores(kl):
                    s0 = s0_of(kl) if causal else 0
                    c0 = s0 * P
                    sm = kl // NQB
                    sc = scps.tile([P, 2 * OWN], f32, tag="sc")
                    for hh in range(2):
                        hb = hh * HD
                        nc.tensor.matmul(
                            sc[:, hh * OWN + c0:(hh + 1) * OWN],
                            kt[hb:hb + HD, kl * P:(kl + 1) * P],
                            qt[hb:hb + HD, c0:OWN],
                            start=True, stop=True)
                    pb = pbp.tile([P, 2 * OWN], bf16, tag="pb")
                    scv = sc[:].rearrange("p (h q) -> p h q", q=OWN)
                    pbv = pb[:].rearrange("p (h q) -> p h q", q=OWN)
                    nc.scalar.activation(pbv[:, :, c0:], scv[:, :, c0:],
                                         AF.Exp, scale=exp_scale)
                    if causal:
                        # multiplicative mask {0, 1, tril} on the own-block
                        # s = kl//4 (zeroed weights vanish from AV and the
                        # ones-column denominator alike)
                        for hh in range(2):
                            nc.vector.tensor_mul(
                                pbv[:, hh, sm * P:(sm + 1) * P],
                                pbv[:, hh, sm * P:(sm + 1) * P],
                                maskt[:, kl * P:(kl + 1) * P])
                    pbs[kl] = (pb, c0)

                def emit_av(kl, ki):
                    pb, c0 = pbs.pop(kl)
                    vbase = kl * PAIRS * 130 + pair * 130
                    nc.tensor.matmul(
                        avtA[0:65, c0:], vsb[:, vbase:vbase + 65],
                        pb[:, c0:OWN],
                        start=(ki == 0), stop=(ki == KB - 1),
                        skip_group_check=True)
                    nc.tensor.matmul(
                        avtB[0:65, c0:], vsb[:, vbase + 65:vbase + 130],
                        pb[:, OWN + c0:2 * OWN],
                        start=(ki == 0), stop=(ki == KB - 1),
                        skip_group_check=True)

                emit_scores(0)
                for ki in range(KB):
                    if ki + 1 < KB:
                        emit_scores(ki + 1)
                    emit_av(ki, ki)
                    if fill and (ki % 2 == 1 or ki == 2):
                        fill.pop(0)()
                while fill:
                    fill.pop(0)()

                # normalize by denominator row 64: both reciprocals issue
                # first (independent DVE work), then the PE broadcasts, then
                # the evict/mul chain; head B leads so its partition-shift
                # DMA overlaps head A's tail
                recB = normp.tile([P, OWN], f32r, tag="rec", name="recB")
                nc.vector.reciprocal(recB[64:65, :], avtB[64:65, :])
                recA = normp.tile([P, OWN], f32r, tag="rec", name="recA")
                nc.vector.reciprocal(recA[64:65, :], avtA[64:65, :])
                bcB = linps.tile([P, 512], f32, tag="lin")
                nc.tensor.matmul(bcB[0:HD, :], onesr[64:65, :],
                                 recB[64:65, :], start=True, stop=True)
                bcA = linps.tile([P, 512], f32, tag="lin")
                nc.tensor.matmul(bcA[0:HD, :], onesr[64:65, :],
                                 recA[64:65, :], start=True, stop=True)
                bcsB = normp.tile([P, OWN], bf16, tag="bcs", name="bcsB")
                nc.vector.tensor_copy(bcsB[0:HD, :], bcB[0:HD, :])
                tmpb = tmpp.tile([P, OWN], avdt, tag="tmpb")
                nc.vector.tensor_mul(tmpb[0:HD, :], avtB[0:HD, :],
                                     bcsB[0:HD, :])
                # partition shift 0:64 -> 64:128 (only DMA can do this)
                nc.sync.dma_start(
                    avall[HD:P, pair * OWN:(pair + 1) * OWN], tmpb[0:HD, :])
                bcsA = normp.tile([P, OWN], bf16, tag="bcs", name="bcsA")
                nc.vector.tensor_copy(bcsA[0:HD, :], bcA[0:HD, :])
                nc.vector.tensor_mul(
                    avall[0:HD, pair * OWN:(pair + 1) * OWN],
                    avtA[0:HD, :], bcsA[0:HD, :])

            def attention_phase(wk_dram, wq_dram, rhs_of, xT, vsb, causal,
                                which, kdt, exp_scale, avdt):
                xT_cur[0] = xT
                avall = avkp.tile([P, PAIRS * OWN], avdt, tag="avall",
                                  name=f"avall_{which}")
                steps, st = kq_steps(0, which, wk_dram, wq_dram, rhs_of, kdt)
                for step in steps:
                    step()
                for pair in range(PAIRS):
                    kt, qt = st['kt'], st['qt']
                    if pair + 1 < PAIRS:
                        nsteps, st = kq_steps(pair + 1, which, wk_dram,
                                              wq_dram, rhs_of, kdt)
                    else:
                        nsteps = []
                    emit_attention(pair, qt, kt, vsb, causal, nsteps, avall,
                                   exp_scale, avdt)
                return avall

            def emit_oproj_residual(wo_dram, bo_t, avall, res_tiles, which,
                                    ev_scale, avdt, stats_to=None):
                """res += transpose(Wo^T @ AV + bo); the transpose of chunk
                m-1 is emitted after chunk m's matmuls so the PE never waits
                on the ACT bias-evict. stats_to: optional per-s st6 tiles --
                the next LayerNorm's bn_stats run chunk-wise here, off the
                critical path."""
                avv = avall[:].rearrange("p (r t) -> p r t", t=OWN)
                pend = []

                def flush(m_out):
                    ev = pend.pop(0)
                    ps2 = linps.tile([P, 512], f32, tag="lin",
                                     name=f"ops2_{which}{m_out}")
                    ps2b = ps2[:].bitcast(bf16)
                    for si in range(NQB):
                        nc.tensor.transpose(ps2b[:, si * P:(si + 1) * P],
                                            ev[:, si * P:(si + 1) * P],
                                            identb[:])
                    for si in range(NQB):
                        nc.vector.tensor_add(
                            res_tiles[si][:, m_out * P:(m_out + 1) * P],
                            ps2b[:, si * P:(si + 1) * P],
                            res_tiles[si][:, m_out * P:(m_out + 1) * P])
                    if stats_to is not None:
                        for si in range(NQB):
                            nc.vector.bn_stats(
                                stats_to[si][:, m_out * 6:(m_out + 1) * 6],
                                res_tiles[si][:, m_out * P:(m_out + 1) * P])

                for m in range(DC):
                    wot = wop.tile([P, PAIRS * P], avdt, tag="wot",
                                   name=f"wot_{which}{m}")
                    nc.sync.dma_start(
                        wot[:], wo_dram[:, m * PAIRS * P:(m + 1) * PAIRS * P])
                    wotv = wot[:].rearrange("p (r c) -> p r c", c=P)
                    ps = linps.tile([P, 512], f32, tag="lin",
                                    name=f"ops_{which}{m}")
                    if avdt == f8:
                        for h in range(2):
                            for q in range(PR2):
                                nc.tensor.matmul(
                                    ps[:, h * 256:(h + 1) * 256],
                                    wotv[:, 2 * q:2 * q + 2, :],
                                    avv[:, 2 * q:2 * q + 2,
                                        h * 256:(h + 1) * 256],
                                    start=(q == 0), stop=(q == PR2 - 1),
                                    perf_mode=DR)
                    else:
                        for pr in range(PAIRS):
                            nc.tensor.matmul(
                                ps[:], wotv[:, pr, :], avv[:, pr, :],
                                start=(pr == 0), stop=(pr == PAIRS - 1))
                    ev = evp.tile([P, OWN], bf16, tag="ev")
                    nc.scalar.activation(ev[:], ps[:], AF.Identity,
                                         bias=bo_t[:, m:m + 1],
                                         scale=ev_scale)
                    pend.append(ev)
                    if m >= 1:
                        flush(m - 1)
                flush(DC - 1)

            def emit_lnq(res_tiles, gbkey, pre=None, dt=f8):
                """LN own tokens + transpose -> [P, DC*OWN] mega-tile."""
                lnq = lnqp.tile([P, DC * OWN], dt, tag="lnq")
                lnqv = lnq[:].rearrange("p (d t) -> p d t", t=OWN)
                for s in range(NQB):
                    lnt = emit_ln(res_tiles[s][:],
                                  pre_st6=None if pre is None else pre[s])
                    emit_transposes(lnt, lnqv[:, :, s * P:(s + 1) * P], gbkey)
                return lnq

            # ================= pipeline =================
            # ---- self-attention (fp8 K/V stream; V of key block 0 runs
            # bf16 so the causal early tokens, which see AV ~= v directly,
            # never meet fp8 V noise; their O-proj is bf16 for the same
            # reason) ----
            xT, vsb = stream_to_xT(x_dec, 1, wv_sa8, "sa", wvb_dram=wv_sa)
            res = []
            for si in range(NQB):
                t = residp.tile([P, D], f32, tag="resid")
                nc.sync.dma_start(t[:], x_own[si * P:(si + 1) * P, :])
                res.append(t)
            lnq1 = emit_lnq(res, 1)
            lnq1v = lnq1[:].rearrange("p (d t) -> p d t", t=OWN)
            lnq1_rhs = lambda i, si: lnq1v[:, 2 * i:2 * i + 2,
                                           si * P:(si + 1) * P]
            av_sa = attention_phase(wk_sa, wq_sa, lnq1_rhs, xT, vsb, True,
                                    "sa", f8, scale * SC_WW, bf16)
            st6_ln2 = [smallp.tile([P, DC * 6], f32, tag="st6s",
                                   name=f"st6ln2_{_s}") for _s in range(NQB)]
            emit_oproj_residual(wo_sa, bo_sa_t, av_sa, res, "sa", SC_W, bf16,
                                stats_to=st6_ln2)

            # ---- cross-attention (keys/values from RAW encoder_x; fully
            # fp8: the 2048-key softmax averaging buries fp8 V noise) ----
            xTe, vsbe = stream_to_xT(x_enc, None, wv_ca, "ca")
            lnq2 = emit_lnq(res, 2, pre=st6_ln2)
            lnq2v = lnq2[:].rearrange("p (d t) -> p d t", t=OWN)
            lnq2_rhs = lambda i, si: lnq2v[:, 2 * i:2 * i + 2,
                                           si * P:(si + 1) * P]
            av_ca = attention_phase(wk_ca, wq_ca, lnq2_rhs, xTe, vsbe, False,
                                    "ca", f8, scale * SC_WW, f8)
            st6_ln3 = [smallp.tile([P, DC * 6], f32, tag="st6s",
                                   name=f"st6ln3_{_s}") for _s in range(NQB)]
            emit_oproj_residual(wo_ca, bo_ca_t, av_ca, res, "ca", SC_WW, f8,
                                stats_to=st6_ln3)

            # ---- FFN ----
            # W1 stage stays bf16: fp8 noise on the hidden would random-walk
            # through W2's 4096-term contraction into ~3% output error. The
            # Relu evict inflates the fp8 hidden by RSCALE (b1 comes in
            # host-premultiplied) to dodge fp8 subnormals near zero.
            lnq3 = emit_lnq(res, 3, pre=st6_ln3, dt=bf16)
            lnq3v = lnq3[:].rearrange("p (d t) -> p d t", t=OWN)
            rT = rtp.tile([P, FC * OWN], bf16, tag="rT")
            rTv = rT[:].rearrange("p (f t) -> p f t", t=OWN)
            for f in range(FC):
                w1t = wp.tile([P, DC * P], bf16, tag="wx", name=f"w1t{f}")
                nc.sync.dma_start(
                    w1t[:], w1[:, f * DC * P:(f + 1) * DC * P])
                ps = linps.tile([P, 512], f32, tag="lin")
                if f < 2:
                    # per-s-block rhs: starts as soon as lnq3's first block
                    # is transposed instead of waiting for all four
                    for si in range(NQB):
                        for d in range(DC):
                            nc.tensor.matmul(
                                ps[:, si * P:(si + 1) * P],
                                w1t[:, d * P:(d + 1) * P],
                                lnq3v[:, d, si * P:(si + 1) * P],
                                start=(d == 0), stop=(d == DC - 1))
                else:
                    for d in range(DC):
                        nc.tensor.matmul(
                            ps[:], w1t[:, d * P:(d + 1) * P],
                            lnq3v[:, d, :],
                            start=(d == 0), stop=(d == DC - 1))
                nc.scalar.activation(rTv[:, f, :], ps[:],
                                     AF.Relu, bias=b1_t[:, f:f + 1],
                                     scale=RSCALE)
            pend2 = []

            def flush2(m_out):
                ev = pend2.pop(0)
                ps2 = linps.tile([P, 512], f32, tag="lin",
                                 name=f"fps2_{m_out}")
                ps2b = ps2[:].bitcast(bf16)
                for si in range(NQB):
                    nc.tensor.transpose(ps2b[:, si * P:(si + 1) * P],
                                        ev[:, si * P:(si + 1) * P],
                                        identb[:])
                for si in range(NQB):
                    nc.vector.tensor_add(res[si][:, m_out * P:(m_out + 1) * P],
                                         ps2b[:, si * P:(si + 1) * P],
                                         res[si][:, m_out * P:(m_out + 1) * P])
                for si in range(NQB):
                    nc.sync.dma_start(
                        out[si * P:(si + 1) * P, m_out * P:(m_out + 1) * P],
                        res[si][:, m_out * P:(m_out + 1) * P])

            for m in range(DC):
                w2t = w2p.tile([P, FC * P], bf16, tag="w2t", name=f"w2t{m}")
                nc.sync.dma_start(
                    w2t[:], w2[:, m * FC * P:(m + 1) * FC * P])
                ps = linps.tile([P, 512], f32, tag="lin", name=f"fps_{m}")
                for fi in range(FC):
                    nc.tensor.matmul(
                        ps[:], w2t[:, fi * P:(fi + 1) * P],
                        rTv[:, fi, :],
                        start=(fi == 0), stop=(fi == FC - 1))
                ev = evp.tile([P, OWN], bf16, tag="ev")
                nc.scalar.activation(ev[:], ps[:], AF.Identity,
                                     bias=b2_t[:, m:m + 1],
                                     scale=1.0 / RSCALE)
                pend2.append(ev)
                if m >= 1:
                    flush2(m - 1)
            flush2(DC - 1)

    nc.compile()
    return nc


def own_token_rows(cfg, j):
    return np.concatenate(
        [np.arange(P * (cfg.NQB * s + j), P * (cfg.NQB * s + j) + P)
         for s in range(cfg.NQB)])


def build_masks(cfg, j):
    """[P, KB*P] bf16: block kl = MULTIPLICATIVE mask (0/1/tril) for own
    q-block s=kl//4 vs key block kl (applied to the post-exp weights)."""
    m = np.zeros((cfg.KB, P, P), np.float32)
    for kl in range(cfg.KB):
        s = kl // cfg.NQB
        g = cfg.NQB * s + j                   # own block's global index
        if kl < g:
            m[kl] = 1.0
        elif kl == g:
            kidx = np.arange(P)[:, None]
            qidx = np.arange(P)[None, :]
            m[kl] = np.where(kidx <= qidx, 1.0, 0.0)
        # kl > g: fully masked -> 0
    out = m.transpose(1, 0, 2).reshape(P, -1)
    return np.ascontiguousarray(out).astype(ml_dtypes.bfloat16)


def _f8w(v):
    """Inflate by WSCALE and quantize to fp8e4m3 (see module docstring)."""
    return np.ascontiguousarray(v * WSCALE).astype(NP_F8)


def _cast(v, fp8, inflate=False):
    if fp8:
        return _f8w(v)
    if inflate:
        v = v * WSCALE
    return np.ascontiguousarray(v).astype(ml_dtypes.bfloat16)


def _pack_pair_proj(w, cfg, fp8=True):
    """[H, D, HD] -> [128, PAIRS*DC*128] (fp8 x WSCALE, or bf16)."""
    w = np.asarray(w, np.float32)
    cat = np.stack([np.concatenate([w[2 * p], w[2 * p + 1]], axis=1)
                    for p in range(cfg.PAIRS)])          # [PR, D, 128]
    v = cat.reshape(cfg.PAIRS, cfg.DC, P, P)             # [PR, d, p, c]
    v = v.transpose(2, 0, 1, 3).reshape(P, -1)           # [p, PR*d*c]
    return _cast(v, fp8)


def _pack_v(w, cfg, fp8=True, inflate=False):
    """[H, D, HD] -> [128, VCH*DC*512] (fp8 x WSCALE, or bf16)."""
    w = np.asarray(w, np.float32)
    vall = w.transpose(1, 0, 2).reshape(cfg.D, cfg.H * HD)   # [D, H*64]
    v = vall.reshape(cfg.DC, P, cfg.VCH, 512)                # [d, p, nch, c]
    v = v.transpose(1, 2, 0, 3).reshape(P, -1)               # [p, nch*d*c]
    return _cast(v, fp8, inflate)


def _pack_o(w, cfg, fp8=True):
    """[D, D] -> [128, DC*PAIRS*128] (fp8 x WSCALE, or bf16)."""
    w = np.asarray(w, np.float32)
    v = w.reshape(cfg.PAIRS, P, cfg.DC, P)               # [pr, p, m, c]
    v = v.transpose(1, 2, 0, 3).reshape(P, -1)           # [p, m*pr*c]
    return _cast(v, fp8)


def _pack_w1(w, cfg):
    """[D, FF] -> [128, FC*DC*128] bf16."""
    w = np.asarray(w, np.float32)
    v = w.reshape(cfg.DC, P, cfg.FC, P)                  # [d, p, f, c]
    v = v.transpose(1, 2, 0, 3).reshape(P, -1)           # [p, f*d*c]
    return np.ascontiguousarray(v).astype(ml_dtypes.bfloat16)


def _pack_w2(w, cfg):
    """[FF, D] -> [128, DC*FC*128] bf16."""
    w = np.asarray(w, np.float32)
    v = w.reshape(cfg.FC, P, cfg.DC, P)                  # [fi, p, m, c]
    v = v.transpose(1, 2, 0, 3).reshape(P, -1)           # [p, m*fi*c]
    return np.ascontiguousarray(v).astype(ml_dtypes.bfloat16)


def prep_core_inputs(cfg, inputs, core):
    """Host-side slicing/packing for one core."""
    b, j = core // 4, core % 4
    a = lambda x: np.asarray(x)
    f32c = lambda x: np.ascontiguousarray(a(x), dtype=np.float32)
    rows = own_token_rows(cfg, j)
    return {
        "x_dec": f32c(a(inputs["decoder_x"])[b]),
        "x_enc": f32c(a(inputs["encoder_x"])[b]),
        "x_own": f32c(a(inputs["decoder_x"])[b][rows]),
        "wq_sa": _pack_pair_proj(inputs["Wq_sa"], cfg),
        "wk_sa": _pack_pair_proj(inputs["Wk_sa"], cfg),
        "wv_sa": _pack_v(inputs["Wv_sa"], cfg, fp8=False, inflate=True),
        "wv_sa8": _pack_v(inputs["Wv_sa"], cfg),
        "wo_sa": _pack_o(inputs["Wo_sa"], cfg, fp8=False),
        "bo_sa": f32c(inputs["bo_sa"]),
        "wq_ca": _pack_pair_proj(inputs["Wq_ca"], cfg),
        "wk_ca": _pack_pair_proj(inputs["Wk_ca"], cfg),
        "wv_ca": _pack_v(inputs["Wv_ca"], cfg),
        "wo_ca": _pack_o(inputs["Wo_ca"], cfg),
        "bo_ca": f32c(inputs["bo_ca"]),
        "w1": _pack_w1(inputs["W1"], cfg),
        "b1": f32c(inputs["b1"]) * RSCALE,
        "w2": _pack_w2(inputs["W2"], cfg),
        "b2": f32c(inputs["b2"]),
        "masks": build_masks(cfg, j),
    }, rows


def gb_trivial(inputs):
    return all(np.allclose(np.asarray(inputs[g]), 1.0)
               for g in ("g1", "g2", "g3")) and \
           all(np.allclose(np.asarray(inputs[b]), 0.0)
               for b in ("be1", "be2", "be3"))


def run(inputs, trace=False, **rk):
    """Build + run on 8 cores; returns (full_output, BassKernelResults)."""
    from concourse.bass_utils import run_bass_kernel_spmd

    cfg = Cfg()
    with_gb = not gb_trivial(inputs)
    nc = build_nc(cfg, with_gb)

    in_maps, rows_all = [], []
    for core in range(8):
        im, rows = prep_core_inputs(cfg, inputs, core)
        if with_gb:
            for n in ("g1", "be1", "g2", "be2", "g3", "be3"):
                im[n] = np.ascontiguousarray(np.asarray(inputs[n]),
                                             dtype=np.float32)
        in_maps.append(im)
        rows_all.append(rows)

    res = run_bass_kernel_spmd(nc, in_maps, list(range(8)), trace=trace, **rk)
    full = np.zeros((2, cfg.T, cfg.D), np.float32)
    for core in range(8):
        full[core // 4][rows_all[core]] = res.results[core]["out"]
    return full, res


def kernel(**inputs) -> np.ndarray:
    return run(inputs)[0]
